# revision 25
# baseline (speedup 1.0000x reference)
"""Trainium2 Bass kernel for AttentionalPlanarRemapping.

out[n,c,h,w] = sum_d softmax(atts[n,c,:])[d] * images[n,d,h,w]

Per-sample: W = softmax(atts[n]) [C,C]; out[n] = W @ images[n].reshape(C, H*W).
Sharding: data-parallel over N across 8 cores (4 samples per core).

Host side (inside kernel()): atts is cast to fp16 and transposed per sample
(attsT[n] = atts[n].T, layout [d, c]) so the contraction dim d lands on
partitions (matmul lhsT layout); images are fp16; output fp16 -> f32.
Everything fp16, not fp8: measured fp8 pipeline error (2-3e-2) exceeds the
2e-2 gate, fp16 is ~1.5e-3. DMA/core = 10MB, warm-PE = ~29.3us, balanced.

Device-side structure per core (measured-trace-driven):
  * Warmup: WARMUP_MMS dummy matmuls on a memset tile keep the PE busy from
    ~7.5us so the HAM clock ramp (1.2->2.4 GHz after ~3.4us of sustained
    busy) completes while sample-0 input DMAs are still in flight. The
    count is tuned so dummies end right when the input stream turns dense.
  * Sample 0 loads are chunked (a00, x00 on a second queue, a-rest, then
    x1..x3 separately) so matmuls can start as chunks land; samples 1-3
    are prefetched as whole-sample DMAs one sample ahead.
  * Matmuls per (sample, kc): psum [128, 1024] (2 banks); per kd a pair of
    N=512 matmuls plus a tiny N=1 matmul (rhs = ones column) into a
    separate 1-bank s-psum tile. The tiny matmul reuses the already-loaded
    PE weights (measured ~4ns) and accumulates the softmax denominator
    directly in per-partition layout.
  * Eviction: DVE reciprocal of s right at group stop, then the scaled
    psum->sbuf fp16 copy (DVE tensor_scalar_mul) deferred by one kc so
    reciprocals never queue behind 1.3us multiplies on the DVE FIFO; the
    last sample evicts immediately. Stores ride gpsimd (SWDGE); o-tiles
    are 4-deep per parity because SWDGE store-completion semaphores lag
    several us behind issue when load traffic is in flight.
  * ACT does only exp (no activation-table thrash) + the final half-tile;
    the last tile splits ACT+DVE with two stores on idle HWDGE queues.
"""

import numpy as np
from contextlib import ExitStack

import concourse.bass as bass
import concourse.mybir as mybir
import concourse.tile as tile
from concourse import bacc
from concourse.bass_utils import run_bass_kernel_spmd

N, C, H, W = 32, 512, 32, 32
HW = H * W                      # 1024
NCORES = 8
NPC = N // NCORES               # 4 samples per core
P = 128
KC = C // P                     # 4 blocks over output channel c
KD = C // P                     # 4 chunks over contraction d
NT = 512                        # matmul moving free dim (one PSUM bank of f32)
NHT = HW // NT                  # 2
WARMUP_MMS = 9

F32 = mybir.dt.float32
F16 = mybir.dt.float16
AF = mybir.ActivationFunctionType


def build_nc():
    nc = bacc.Bacc("TRN2", target_bir_lowering=False, debug=False)

    images = nc.dram_tensor("images", [NPC, C, HW], F16, kind="ExternalInput").ap()
    attsT = nc.dram_tensor("attsT", [NPC, C, C], F16, kind="ExternalInput").ap()
    out = nc.dram_tensor("out", [NPC, C, HW], F16, kind="ExternalOutput").ap()

    with ExitStack() as ctx:
        tc = ctx.enter_context(tile.TileContext(nc))

        const_pool = ctx.enter_context(tc.tile_pool(name="const", bufs=1))
        ones = const_pool.tile([P, NT], F16)

        # sample-0 fast path: per-chunk tiles
        a0_pool = ctx.enter_context(tc.tile_pool(name="a0", bufs=1))
        x0_pool = ctx.enter_context(tc.tile_pool(name="x0", bufs=1))
        # samples 1..3: whole-sample tiles, double buffered
        a_pool = ctx.enter_context(tc.tile_pool(name="a", bufs=2))
        x_pool = ctx.enter_context(tc.tile_pool(name="x", bufs=2))
        e_pool = ctx.enter_context(tc.tile_pool(name="e", bufs=2))
        e0_pool = ctx.enter_context(tc.tile_pool(name="e0", bufs=1))
        o_pool = ctx.enter_context(tc.tile_pool(name="o", bufs=4))
        r_pool = ctx.enter_context(tc.tile_pool(name="r", bufs=2))
        mm_psum = ctx.enter_context(tc.tile_pool(name="mmp", bufs=3, space="PSUM"))
        s_psum = ctx.enter_context(tc.tile_pool(name="sp", bufs=1, space="PSUM"))
        wu_psum = ctx.enter_context(tc.tile_pool(name="wup", bufs=1, space="PSUM"))

        # ---- t=0: consts + PE warmup (overlaps the first loads) ----
        nc.gpsimd.memset(ones[:], 1.0)
        wu_ps = wu_psum.tile([P, NT], F32, name="wu", space="PSUM")
        for i in range(WARMUP_MMS):
            nc.tensor.matmul(
                wu_ps[:], lhsT=ones[:, 0:P], rhs=ones[:], start=True, stop=True
            )

        # ---- sample 0: fast first-matmul path, 4 load DMAs ----
        # (each dma_start costs ~750ns of HWDGE sequencer issue time, so
        # chunk only the kd=0 slices; the rest go as one DMA apiece)
        a00 = a0_pool.tile([P, C], F16, name="a00", tag="a00")
        nc.sync.dma_start(a00[:], attsT[0][0:P])
        x00 = x0_pool.tile([P, HW], F16, name="x00", tag="x00")
        nc.scalar.dma_start(x00[:], images[0][0:P])
        a0r = a0_pool.tile([P, KD - 1, C], F16, name="a0r", tag="a0r")
        nc.sync.dma_start(
            a0r[:], attsT[0][P:C].rearrange("(kd p) c -> p kd c", p=P)
        )
        x0r = x0_pool.tile([P, KD - 1, HW], F16, name="x0r", tag="x0r")
        nc.sync.dma_start(
            x0r[:], images[0][P:C].rearrange("(kd p) f -> p kd f", p=P)
        )
        e00 = e0_pool.tile([P, C], F16, name="e00", tag="e00")
        nc.scalar.activation(e00[:], a00[:], AF.Exp, bias=0.0, scale=1.0)
        e0r = e0_pool.tile([P, KD - 1, C], F16, name="e0r", tag="e0r")
        for k in range(KD - 1):
            nc.scalar.activation(
                e0r[:, k], a0r[:, k], AF.Exp, bias=0.0, scale=1.0
            )
        e0ap = lambda kd: e00[:] if kd == 0 else e0r[:, kd - 1]
        x0ap = lambda kd: x00[:] if kd == 0 else x0r[:, kd - 1]

        def prep(n):
            """Whole-sample loads + exp for sample n >= 1 (prefetched)."""
            a_t = a_pool.tile([P, KD, C], F16, name=f"a{n}", tag="a")
            nc.sync.dma_start(
                a_t[:], attsT[n].rearrange("(kd p) c -> p kd c", p=P)
            )
            half = C // 2
            x_lo = x_pool.tile([P, KD // 2, HW], F16, name=f"xl{n}", tag="xl")
            nc.sync.dma_start(
                x_lo[:], images[n][0:half].rearrange("(kd p) f -> p kd f", p=P)
            )
            x_hi = x_pool.tile([P, KD // 2, HW], F16, name=f"xh{n}", tag="xh")
            nc.sync.dma_start(
                x_hi[:], images[n][half:C].rearrange("(kd p) f -> p kd f", p=P)
            )
            e_t = e_pool.tile([P, KD, C], F16, name=f"e{n}", tag="e")
            for k in range(KD):
                nc.scalar.activation(
                    e_t[:, k], a_t[:, k], AF.Exp, bias=0.0, scale=1.0
                )
            return e_t, (x_lo, x_hi)

        def dummy_mm():
            """Filler matmul: keeps the PE busy-window unbroken during
            sample-0 data waits so the HAM clock ramp completes early."""
            nc.tensor.matmul(
                wu_ps[:], lhsT=ones[:, 0:P], rhs=ones[:], start=True, stop=True
            )

        pend = []

        def evict(n, kc, ps, r_t):
            """Scaled psum->sbuf eviction + store (deferred one kc so DVE
            reciprocals never queue behind the big multiplies)."""
            rows = out[n][kc * P : (kc + 1) * P]
            if not (n == NPC - 1 and kc == KC - 1):
                o_t = o_pool.tile([P, HW], F16, name=f"o{n}_{kc}", tag=f"o{kc % 2}")
                nc.vector.tensor_scalar_mul(o_t[:], ps[:, 0:HW], r_t[:, 0:1])
                nc.gpsimd.dma_start(rows, o_t[:])
            else:
                # tail: halves on ACT+DVE; stores issue on sync/gpsimd whose
                # sequencers are NOT doing the evicting (a store's ~0.65us
                # DIRECT2D on the evicting engine's own queue serializes)
                oa = o_pool.tile([P, NT], F16, name="oa", tag="oa")
                ob = o_pool.tile([P, NT], F16, name="ob", tag="ob")
                nc.scalar.mul(oa[:], ps[:, 0:NT], r_t[:, 0:1])
                nc.vector.tensor_scalar_mul(ob[:], ps[:, NT : 2 * NT], r_t[:, 0:1])
                rows3 = rows.rearrange("c (h f) -> c h f", h=NHT)
                nc.scalar.dma_start(rows3[:, 0], oa[:])
                nc.sync.dma_start(rows3[:, 1], ob[:])

        def compute(n, eap, xap):
            """eap(kd) -> [P, C] E chunk AP; xap(kd) -> [P, HW] X chunk AP."""
            for kc in range(KC):
                ps = mm_psum.tile(
                    [P, 2 * NT], F32, name=f"ps{n}_{kc}", tag="ps", space="PSUM"
                )
                s_t = s_psum.tile(
                    [P, NT], F32, name=f"s{n}_{kc}", tag="s", space="PSUM"
                )
                for kd in range(KD):
                    w_ap = eap(kd)[:, kc * P : (kc + 1) * P]
                    nc.tensor.matmul(
                        ps[:, 0:NT],
                        lhsT=w_ap,
                        rhs=xap(kd)[:, 0:NT],
                        start=(kd == 0),
                        stop=(kd == KD - 1),
                    )
                    nc.tensor.matmul(
                        ps[:, NT : 2 * NT],
                        lhsT=w_ap,
                        rhs=xap(kd)[:, NT : 2 * NT],
                        start=(kd == 0),
                        stop=(kd == KD - 1),
                    )
                    nc.tensor.matmul(
                        s_t[:, 0:1],
                        lhsT=w_ap,
                        rhs=ones[:, 0:1],
                        start=(kd == 0),
                        stop=(kd == KD - 1),
                    )
                r_t = r_pool.tile([P, 1], F32, name=f"r{n}_{kc}", tag="r")
                nc.vector.reciprocal(r_t[:], s_t[:, 0:1])
                pend.append((n, kc, ps, r_t))
                keep = 0 if n == NPC - 1 else 1
                while len(pend) > keep:
                    evict(*pend.pop(0))

        staged = None
        for n in range(NPC):
            if n + 1 < NPC:
                nxt = prep(n + 1)
            else:
                nxt = None
            if n == 0:
                compute(0, e0ap, x0ap)
            else:
                e_t, (x_lo, x_hi) = staged
                compute(
                    n,
                    lambda kd: e_t[:, kd],
                    lambda kd: x_lo[:, kd] if kd < KD // 2 else x_hi[:, kd - KD // 2],
                )
            staged = nxt

    nc.compile()
    return nc


_NC_CACHE = None


def _get_nc():
    global _NC_CACHE
    if _NC_CACHE is None:
        _NC_CACHE = build_nc()
    return _NC_CACHE


def run(in_maps, **kwargs):
    """Run the SPMD kernel on cores 0..7. in_maps: one dict per core."""
    nc = _get_nc()
    return run_bass_kernel_spmd(nc, in_maps, core_ids=list(range(NCORES)), **kwargs)


def make_in_maps(images: np.ndarray, atts: np.ndarray):
    images = np.asarray(np.asarray(images, dtype=np.float32), dtype=np.float16)
    atts = np.asarray(atts, dtype=np.float32)
    assert images.shape == (N, C, H, W), images.shape
    assert atts.shape == (N, C, C), atts.shape
    img_s = np.ascontiguousarray(images.reshape(NCORES, NPC, C, HW))
    # per-sample transpose: attsT[n] = atts[n].T  (layout [d, c]), fp16
    attsT = np.ascontiguousarray(atts.transpose(0, 2, 1).astype(np.float16)).reshape(
        NCORES, NPC, C, C
    )
    return [{"images": img_s[i], "attsT": attsT[i]} for i in range(NCORES)]


def kernel(images: np.ndarray, atts: np.ndarray) -> np.ndarray:
    in_maps = make_in_maps(images, atts)
    res = run(in_maps)
    outs = [res.results[i]["out"] for i in range(NCORES)]
    full = np.concatenate(outs, axis=0).reshape(N, C, H, W)
    return full.astype(np.float32)
